# revision 20
# baseline (speedup 1.0000x reference)
# Bidirectional cross-attention (talking heads) on 8 trn2 cores.
#
# Sharding: core c -> batch c//2, query-row half c%2 (audio rows). Each core
# computes the full attention for its 512 query rows against all 1024 text rows.
#
# Per-core dataflow (all matmuls bf16, fp32 accumulate):
#   LN(audio), LN(text) in [row, d] layout -> PE-transpose -> z^T [d, row]
#   qk^T = (g*scale*W_qk)^T @ z_a^T        [inner, 512]
#   v^T  = (g*W_v)^T @ z_t^T               [inner, 1024];  v = transpose(v^T)
#   Attention runs in 4 i-chunks of 128 (sim+exp in 2 super-chunks of 256 so
#   the stationary sim weights stream n=256):
#     sim^T[j,i] per head; e = exp(sim^T) -> et [jl, g, i]
#     DMA partition-shuffle via DRAM staging -> PK [(jt,g), jl, i]
#     Z[g,i] via indicator matmul; PK *= 1/Z
#     talking-heads: block-diag W -> 128x128 stationary matmul over PK
#     DMA shuffle back -> MX [jl, jt, h, i]
#     out2^T[(h,d), i] = v^T-tiles @ MX ; out = out2^T^T @ W_out (+b_out host)
#   The chunk pipeline is software-skewed so the staging DMAs, the softmax
#   normalize (DVE) and the PSUM evacuations (Pool/DVE) hide under PE work.
import numpy as np
import ml_dtypes
from contextlib import ExitStack

import concourse.bass as bass
import concourse.tile as tile
from concourse import mybir
from concourse.bass_utils import run_bass_kernel_spmd

BF16 = mybir.dt.bfloat16
F32 = mybir.dt.float32
AF = mybir.ActivationFunctionType
OP = mybir.AluOpType

HEADS, DH, D = 16, 64, 1024
I, J = 512, 1024  # per-core audio (query) rows, text rows
IH = 128          # i-chunk for the attention back-half
IHS = 256         # i-super-chunk for sim+exp (n=256 streams)
NQ = I // IH      # 4
NS = I // IHS     # 2
EPS = 1e-5
N_CORES = 8


def _layernorm_to_zT(nc, pools, x_src, zT, col0, eps_tile, ident, ld_eng):
    """DMA a [128, D] row-tile, layernorm core (no affine), transpose into
    zT[:, dt, col0:col0+128] (bf16, feature dim on partitions)."""
    xpool, stats, zbpool, tps = pools
    x = xpool.tile([128, D], F32)
    ld_eng.dma_start(out=x, in_=x_src)
    st = stats.tile([128, 2, 6], F32, tag="st")
    nc.vector.bn_stats(out=st[:, 0, :], in_=x[:, 0:512])
    nc.vector.bn_stats(out=st[:, 1, :], in_=x[:, 512:1024])
    mv = stats.tile([128, 2], F32, tag="mv")
    nc.vector.bn_aggr(out=mv, in_=st)
    rstd = stats.tile([128, 1], F32, tag="rstd")
    nc.scalar.activation(out=rstd, in_=mv[:, 1:2], func=AF.Sqrt, bias=eps_tile,
                         scale=1.0)
    nc.vector.reciprocal(out=rstd, in_=rstd)
    zb = zbpool.tile([128, D], BF16)
    nc.vector.tensor_scalar(out=zb, in0=x, scalar1=mv[:, 0:1], scalar2=rstd,
                            op0=OP.subtract, op1=OP.mult)
    for dt_ in range(8):
        ps = tps.tile([128, 128], BF16)
        nc.tensor.transpose(ps, zb[:, dt_ * 128:(dt_ + 1) * 128], ident)
        if dt_ % 2 == 0:
            nc.scalar.copy(out=zT[:, dt_, col0:col0 + 128], in_=ps)
        else:
            nc.vector.tensor_copy(out=zT[:, dt_, col0:col0 + 128], in_=ps)


def _legalize_dma_waits(nc):
    """This container's walrus only supports ONE sync-wait on dynamic DMA
    instructions (PSEUDO_DMA_DIRECT2D).  Tile attaches several.  Move the
    excess onto EventSemaphore instructions inserted just before each DMA on
    the same issuing engine (evsems hold up to 2 waits each)."""
    import bass_rust as br

    def cap_of(ins):
        return 2 if type(ins).__name__ == "InstEventSemaphore" else 1

    n_fixed = 0
    for f in nc.m.functions:
        for blk in f.blocks:
            il = blk.instructions
            if not any(getattr(i, "sync_info", None)
                       and len(i.sync_info.on_wait) > cap_of(i) for i in il):
                continue
            newlist = []
            for ins in il:
                si = getattr(ins, "sync_info", None)
                cap = cap_of(ins)
                if si is not None and len(si.on_wait) > cap:
                    waits = list(si.on_wait)
                    extra, keep = waits[:-cap], waits[-cap:]
                    for k in range(0, len(extra), 2):
                        ev = mybir.InstEventSemaphore(
                            name=f"{ins.name}-wev{k}", ins=[], outs=[])
                        ev.engine = ins.engine
                        ev.sync_info = br.SyncInfo(on_wait=extra[k:k + 2],
                                                   on_update=[])
                        newlist.append(ev)
                    si.on_wait = keep
                    n_fixed += 1
                newlist.append(ins)
            blk.instructions = newlist
    return n_fixed


def build_nc(legalize=True, stop_after=99):
    nc = bass.Bass()
    audio = nc.declare_dram_parameter("audio", [I, D], F32, isOutput=False)
    text = nc.declare_dram_parameter("text", [J, D], F32, isOutput=False)
    w1 = nc.declare_dram_parameter("w1", [D, D], BF16, isOutput=False)
    w2 = nc.declare_dram_parameter("w2", [D, D], BF16, isOutput=False)
    wout = nc.declare_dram_parameter("wout", [D, D], BF16, isOutput=False)
    c1 = nc.declare_dram_parameter("c1", [128, 8], F32, isOutput=False)
    c2 = nc.declare_dram_parameter("c2", [128, 8], F32, isOutput=False)
    wbig = nc.declare_dram_parameter("wbig", [128, 128], BF16, isOutput=False)
    sind = nc.declare_dram_parameter("sind", [128, 16], BF16, isOutput=False)
    ident = nc.declare_dram_parameter("ident", [128, 128], BF16, isOutput=False)
    out = nc.declare_dram_parameter("out", [I, D], F32, isOutput=True)

    with tile.TileContext(nc) as tc, ExitStack() as ctx:
        singles = ctx.enter_context(tc.tile_pool(name="singles", bufs=1))
        persist = ctx.enter_context(tc.tile_pool(name="persist", bufs=1))

        # --- resident constants/weights ---
        WOSB = singles.tile([128, 8, D], BF16)
        nc.sync.dma_start(out=WOSB, in_=wout[:, :].rearrange("(t p) n -> p t n", p=128))
        WBIGSB = singles.tile([128, 128], BF16)
        nc.sync.dma_start(out=WBIGSB, in_=wbig[:, :])
        SINDSB = singles.tile([128, 16], BF16)
        nc.sync.dma_start(out=SINDSB, in_=sind[:, :])
        IDENT = singles.tile([128, 128], BF16)
        nc.sync.dma_start(out=IDENT, in_=ident[:, :])
        C1SB = singles.tile([128, 8], F32)
        nc.sync.dma_start(out=C1SB, in_=c1[:, :])
        C2SB = singles.tile([128, 8], F32)
        nc.sync.dma_start(out=C2SB, in_=c2[:, :])
        eps_tile = singles.tile([128, 1], F32)
        nc.vector.memset(eps_tile, EPS)

        # --- persistent activations ---
        QKT = persist.tile([128, 8, I], BF16)    # qk^T: [d-part, inner-tile, i]
        VN = persist.tile([128, 8, D], BF16)     # v:    [j-part, j-tile, inner]
        OUT2T = persist.tile([128, 8, IH], BF16)  # out2^T per chunk

        # ================= Phase A: LN + transposes + projections ============
        vtctx = tc.tile_pool(name="vt", bufs=1)
        vtpool = vtctx.__enter__()
        VT = vtpool.tile([128, 8, J], BF16)      # v^T: [d-part, inner-tile, j]
        with tc.tile_pool(name="xp", bufs=3) as xpool, \
             tc.tile_pool(name="stats", bufs=4) as stats, \
             tc.tile_pool(name="zb", bufs=3) as zbpool, \
             tc.tile_pool(name="zt", bufs=1) as ztpool, \
             tc.tile_pool(name="tps", bufs=2, space="PSUM") as tps, \
             tc.tile_pool(name="pps", bufs=2, space="PSUM") as pps:
            ZAT = ztpool.tile([128, 8, I], BF16)
            ZTT = ztpool.tile([128, 8, J], BF16)
            W1SB = ztpool.tile([128, 8, D], BF16)
            nc.gpsimd.dma_start(out=W1SB, in_=w1[:, :].rearrange("(t p) n -> p t n", p=128))
            W2SB = ztpool.tile([128, 8, D], BF16)
            nc.gpsimd.dma_start(out=W2SB, in_=w2[:, :].rearrange("(t p) n -> p t n", p=128))
            pools = (xpool, stats, zbpool, tps)
            for it in range(4):
                _layernorm_to_zT(nc, pools, audio[it * 128:(it + 1) * 128, :],
                                 ZAT, it * 128, eps_tile, IDENT, nc.gpsimd)
            # qk^T = W1^T @ z_a^T   [inner, I]  (overlaps text LN loads)
            for jt in range(2):
                _layernorm_to_zT(nc, pools, text[jt * 128:(jt + 1) * 128, :],
                                 ZTT, jt * 128, eps_tile, IDENT, nc.gpsimd)
            for mt in range(8):
                ps = pps.tile([128, I], F32)
                for kt in range(8):
                    nc.tensor.matmul(ps, W1SB[:, kt, mt * 128:(mt + 1) * 128],
                                     ZAT[:, kt, :], start=(kt == 0),
                                     stop=(kt == 7))
                nc.scalar.activation(out=QKT[:, mt, :], in_=ps, func=AF.Identity,
                                     bias=C1SB[:, mt:mt + 1], scale=1.0)
            for jt in range(2, 8):
                _layernorm_to_zT(nc, pools, text[jt * 128:(jt + 1) * 128, :],
                                 ZTT, jt * 128, eps_tile, IDENT, nc.gpsimd)
            # v^T = W2^T @ z_t^T   [inner, J]
            for mt in range(8):
                for nh in range(2):
                    ps = pps.tile([128, 512], F32, tag="vps")
                    for kt in range(8):
                        nc.tensor.matmul(ps, W2SB[:, kt, mt * 128:(mt + 1) * 128],
                                         ZTT[:, kt, nh * 512:(nh + 1) * 512],
                                         start=(kt == 0), stop=(kt == 7))
                    nc.scalar.activation(out=VT[:, mt, nh * 512:(nh + 1) * 512],
                                         in_=ps, func=AF.Identity,
                                         bias=C2SB[:, mt:mt + 1], scale=1.0)
            # v natural layout: transpose VT
            k = 0
            for mt in range(8):
                for jt in range(8):
                    ps = tps.tile([128, 128], BF16, tag="vn", bufs=2)
                    nc.tensor.transpose(ps, VT[:, mt, jt * 128:(jt + 1) * 128],
                                        IDENT)
                    if k % 2 == 0:
                        nc.scalar.copy(out=VN[:, jt, mt * 128:(mt + 1) * 128],
                                       in_=ps)
                    else:
                        nc.vector.tensor_copy(
                            out=VN[:, jt, mt * 128:(mt + 1) * 128], in_=ps)
                    k += 1

        # ================= Phase B: attention, 4-chunk pipeline ==============
        with tc.tile_pool(name="pk", bufs=2) as pkpool, \
             tc.tile_pool(name="mx", bufs=1) as mxpool, \
             tc.tile_pool(name="et", bufs=2) as etpool, \
             tc.tile_pool(name="mxc", bufs=3) as mxcpool, \
             tc.tile_pool(name="zr", bufs=2) as zrpool, \
             tc.tile_pool(name="ob", bufs=2) as obpool, \
             tc.tile_pool(name="stg1", bufs=2, space="DRAM") as stg1pool, \
             tc.tile_pool(name="stg2", bufs=2, space="DRAM") as stg2pool, \
             tc.tile_pool(name="simps", bufs=2, space="PSUM") as simps, \
             tc.tile_pool(name="mixps", bufs=2, space="PSUM") as mixps, \
             tc.tile_pool(name="auxps", bufs=2, space="PSUM") as auxps:

            stg1 = [None] * NQ
            PK = [None] * NQ
            MXT = [None] * NQ

            def sim_superchunk(s, early_pk_load):
                """sim + exp + shuffle1 staging for chunks 2s, 2s+1.  When
                early_pk_load, the PK reload slices are issued per-jt right
                behind the staging write (first superchunk: PK bufs free)."""
                i0 = s * IHS
                for q in (2 * s, 2 * s + 1):
                    stg1[q] = stg1pool.tile([128, 128, IH], BF16, tag="stg1", name="stg1t")
                    if early_pk_load:
                        PK[q] = pkpool.tile([128, 128, IH], BF16, tag="pk", name="pkt")
                for jt in range(8):
                    et = etpool.tile([128, HEADS, IHS], BF16)
                    for t in range(8):  # 2 heads per 2-bank psum tile
                        ps = simps.tile([128, 2, IHS], F32, tag="sim",
                                        padded_shape=[128, 2, 512], name="ps")
                        for r in range(2):
                            h = 2 * t + r
                            hp, hf = h // 2, (h % 2) * 64
                            nc.tensor.matmul(
                                ps[:, r, :],
                                VT[hf:hf + 64, hp, jt * 128:(jt + 1) * 128],
                                QKT[hf:hf + 64, hp, i0:i0 + IHS],
                                skip_group_check=True)
                        nc.scalar.activation(out=et[:, 2 * t:2 * t + 2, :],
                                             in_=ps, func=AF.Exp)
                    # stage both i-halves: stg1[q][(jt,g), jl, i]
                    for half in range(2):
                        q = 2 * s + half
                        dst = stg1[q].rearrange("p jl i -> jl p i")
                        nc.sync.dma_start(
                            out=dst[:, jt * 16:(jt + 1) * 16, :],
                            in_=et[:, :, half * IH:(half + 1) * IH])
                        if early_pk_load:
                            nc.sync.dma_start(
                                out=PK[q][jt * 16:(jt + 1) * 16, :, :],
                                in_=stg1[q][jt * 16:(jt + 1) * 16, :, :])

            def pk_load(q):
                """Sliced PK reload for the second superchunk (waits on the
                mix that frees the PK buffer; issued on the scalar queue which
                is idle outside the sim phases)."""
                PK[q] = pkpool.tile([128, 128, IH], BF16, tag="pk", name="pkt")
                for jt in range(8):
                    nc.scalar.dma_start(
                        out=PK[q][jt * 16:(jt + 1) * 16, :, :],
                        in_=stg1[q][jt * 16:(jt + 1) * 16, :, :])

            def z_stage(q):
                """Z indicator matmul + reciprocal + broadcast."""
                zps = auxps.tile([16, 4, IH], F32, tag="aux", name="zps")
                for cc in range(32):
                    nc.tensor.matmul(zps, SINDSB,
                                     PK[q][:, cc * 4:(cc + 1) * 4, :],
                                     start=(cc == 0), stop=(cc == 31))
                zsb = zrpool.tile([16, IH], F32, tag="zsb")
                nc.vector.tensor_reduce(out=zsb,
                                        in_=zps.rearrange("p a b -> p b a"),
                                        axis=mybir.AxisListType.X, op=OP.add)
                nc.vector.reciprocal(out=zsb, in_=zsb)
                zrb = zrpool.tile([16, IH], BF16, tag="zrb")
                nc.gpsimd.tensor_copy(out=zrb, in_=zsb)
                ZRPK = zrpool.tile([128, IH], BF16, tag="zrpk")
                for sct in range(8):
                    nc.sync.dma_start(out=ZRPK[sct * 16:(sct + 1) * 16, :],
                                      in_=zrb)
                return ZRPK

            def norm_stage(q, ZRPK):
                zb_ap = bass.AP(tensor=ZRPK.tensor, offset=ZRPK.offset,
                                ap=[list(ZRPK.ap[0]), [0, 16], list(ZRPK.ap[1])])
                for cc in range(8):
                    nc.vector.tensor_mul(out=PK[q][:, cc * 16:(cc + 1) * 16, :],
                                         in0=PK[q][:, cc * 16:(cc + 1) * 16, :],
                                         in1=zb_ap)

            def mix_stage(q):
                """talking-heads mix + evac + shuffle2 staging + sliced MX
                reload (each 16-partition slice reloads right after its
                staging write lands)."""
                stg2 = stg2pool.tile([128, 8, HEADS, IH], BF16, tag="stg2")
                MXT[q] = mxpool.tile([128, 8, HEADS, IH], BF16, tag="mx", name="mxt")
                for cg in range(8):
                    mxc = mxcpool.tile([128, 16, IH], BF16)
                    for c4 in range(4):
                        mps = mixps.tile([128, 4, IH], F32)
                        nc.tensor.matmul(
                            mps, WBIGSB,
                            PK[q][:, cg * 16 + c4 * 4:cg * 16 + (c4 + 1) * 4, :])
                        if c4 % 2 == 0:
                            nc.scalar.copy(out=mxc[:, c4 * 4:(c4 + 1) * 4, :],
                                           in_=mps)
                        else:
                            nc.vector.tensor_copy(
                                out=mxc[:, c4 * 4:(c4 + 1) * 4, :], in_=mps)
                    # stg2 [jl, jt, h, i] <- mxc [(jt,h), jl16, i]
                    dst = stg2.rearrange("jl jt h i -> (jt h) jl i")
                    nc.gpsimd.dma_start(
                        out=dst[:, cg * 16:(cg + 1) * 16, :], in_=mxc)
                    nc.sync.dma_start(
                        out=MXT[q][cg * 16:(cg + 1) * 16, :, :, :],
                        in_=stg2[cg * 16:(cg + 1) * 16, :, :, :])

            def av_stage(q):
                for t in range(8):
                    aps = auxps.tile([128, IH], F32, tag="aux", name="aps")
                    for jt in range(8):
                        nc.tensor.matmul(aps[0:64, :],
                                         VN[:, jt, (2 * t) * 64:(2 * t + 1) * 64],
                                         MXT[q][:, jt, 2 * t, :],
                                         start=(jt == 0), stop=(jt == 7),
                                         skip_group_check=True)
                        nc.tensor.matmul(aps[64:128, :],
                                         VN[:, jt, (2 * t + 1) * 64:(2 * t + 2) * 64],
                                         MXT[q][:, jt, 2 * t + 1, :],
                                         start=(jt == 0), stop=(jt == 7),
                                         skip_group_check=True)
                    nc.vector.tensor_copy(out=OUT2T[:, t, :], in_=aps)

            def fin_stage(q):
                r0 = q * IH
                for nh in range(2):
                    fps = auxps.tile([128, 512], F32, tag="aux", name="fps")
                    for kt in range(8):
                        nc.tensor.matmul(fps, OUT2T[:, kt, :],
                                         WOSB[:, kt, nh * 512:(nh + 1) * 512],
                                         start=(kt == 0), stop=(kt == 7))
                    ob = obpool.tile([128, 512], F32)
                    nc.scalar.copy(out=ob, in_=fps)
                    nc.gpsimd.dma_start(
                        out=out[r0:r0 + 128, nh * 512:(nh + 1) * 512], in_=ob)

            # ---- software-pipelined schedule ----
            steps = [
                lambda: sim_superchunk(0, early_pk_load=True),
                lambda: norm_stage(0, z_stage(0)),
                lambda: sim_superchunk(1, early_pk_load=False),
                lambda: norm_stage(1, z_stage(1)),
                lambda: mix_stage(0),
                lambda: pk_load(2),
                lambda: av_stage(0),
                lambda: fin_stage(0),
                lambda: norm_stage(2, z_stage(2)),
                lambda: mix_stage(1),
                lambda: pk_load(3),
                lambda: av_stage(1),
                lambda: fin_stage(1),
                lambda: norm_stage(3, z_stage(3)),
                lambda: mix_stage(2),
                lambda: av_stage(2),
                lambda: fin_stage(2),
                lambda: mix_stage(3),
                lambda: av_stage(3),
                lambda: fin_stage(3),
            ]
            for si, step in enumerate(steps):
                if si >= stop_after:
                    break
                step()
        vtctx.__exit__(None, None, None)
    if legalize:
        _legalize_dma_waits(nc)
    return nc


def _host_prep(text, audio, g_text, b_text, g_audio, b_audio, W_qk, W_v, W_out,
               b_out, W_th):
    bf16 = ml_dtypes.bfloat16
    scale = DH ** -0.5
    w1 = (g_audio[:, None] * W_qk * scale).astype(bf16)
    c1 = (scale * (b_audio @ W_qk)).astype(np.float32)
    w2 = (g_text[:, None] * W_v).astype(bf16)
    c2 = (b_text @ W_v).astype(np.float32)
    wout = W_out.astype(bf16)
    wbig = np.zeros((128, 128), np.float32)
    for s in range(8):
        wbig[s * 16:(s + 1) * 16, s * 16:(s + 1) * 16] = W_th.T
    wbig = wbig.astype(bf16)
    sind = np.tile(np.eye(16, dtype=np.float32), (8, 1)).astype(bf16)
    ident = np.eye(128, dtype=np.float32).astype(bf16)
    # pack [1024] -> [128, 8] with c[p, t] = vec[t*128 + p]
    c1p = np.ascontiguousarray(c1.reshape(8, 128).T)
    c2p = np.ascontiguousarray(c2.reshape(8, 128).T)
    shared = dict(w1=w1, w2=w2, wout=wout, c1=c1p, c2=c2p, wbig=wbig,
                  sind=sind, ident=ident)
    in_maps = []
    for core in range(N_CORES):
        b, half = core // 2, core % 2
        in_maps.append(dict(
            audio=np.ascontiguousarray(audio[b, half * I:(half + 1) * I, :],
                                       dtype=np.float32),
            text=np.ascontiguousarray(text[b], dtype=np.float32),
            **shared))
    return in_maps


_NC = None


def _get_nc():
    global _NC
    if _NC is None:
        _NC = build_nc()
    return _NC


def kernel(text, audio, g_text, b_text, g_audio, b_audio, W_qk, W_v, W_out,
           b_out, W_th, _trace=False):
    text = np.asarray(text, np.float32)
    audio = np.asarray(audio, np.float32)
    in_maps = _host_prep(np.asarray(text, np.float32),
                         np.asarray(audio, np.float32),
                         np.asarray(g_text, np.float32),
                         np.asarray(b_text, np.float32),
                         np.asarray(g_audio, np.float32),
                         np.asarray(b_audio, np.float32),
                         np.asarray(W_qk, np.float32),
                         np.asarray(W_v, np.float32),
                         np.asarray(W_out, np.float32),
                         np.asarray(b_out, np.float32),
                         np.asarray(W_th, np.float32))
    nc = _get_nc()
    res = run_bass_kernel_spmd(nc, in_maps, list(range(N_CORES)), trace=_trace)
    b_ = audio.shape[0]
    full = np.empty((b_, 2 * I, D), np.float32)
    for core in range(N_CORES):
        b, half = core // 2, core % 2
        full[b, half * I:(half + 1) * I, :] = res.results[core]["out"]
    full += np.asarray(b_out, np.float32)[None, None, :]
    if _trace:
        return full, res
    return full
